# revision 1
# baseline (speedup 1.0000x reference)
"""MoE cascaded FFN (top-2, capacity-dispatched) on 8 Trainium2 NeuronCores.

Strategy: expert-parallel, one expert per core (E == n_cores == 8).
  - Host: gating softmax + top-2 + GShard k-major capacity dispatch
    (pure bookkeeping + gathers), pack each expert's tokens as
    dispT [M, CAP] plus pre-layouted weights.
  - Device (SPMD, identical program, per-expert data):
        hT = relu(W1 @ disp_eT + b1)   via  lhsT=W1T tiles, rhs=dispT
        y  = hT.T @ W2                 via  lhsT=hT tiles,  rhs=W2
    Output y [CAP, M] in fp32; all matmul operands bf16 (PE full rate),
    fp32 accumulation in PSUM.
  - Host: gather rows back per (token, k), weight by normalized gates,
    add the (token-dependent) fc2 bias contribution.
"""

import numpy as np
import ml_dtypes

T, M, H, E, K = 8192, 1024, 4096, 8, 2
CAP = 2560
N_CORES = 8

HT = H // 128  # 32 h tiles
KT = M // 128  # 8 contraction tiles for GEMM1

_PROGRAMS = {}
PROFILE = False
LAST_RESULT = None


def _build_program(c_pad):
    import concourse.mybir as mybir
    import concourse.tile as tile
    from concourse import bacc

    bf16 = mybir.dt.bfloat16
    f32 = mybir.dt.float32

    nc = bacc.Bacc("TRN2", target_bir_lowering=False, debug=False,
                   num_devices=N_CORES)

    # c-chunk schedule: 512-wide chunks (full PE efficiency) plus at most one
    # 256-wide remainder
    assert c_pad % 256 == 0 and 256 <= c_pad <= CAP
    chunks = [512] * (c_pad // 512)
    if c_pad % 512:
        chunks.append(256)

    dispT = nc.declare_dram_parameter("dispT", [M, c_pad], bf16,
                                      isOutput=False)
    # w1[ht, p, k*128+f] = fc1_w[e][ht*128+f, k*128+p]
    w1 = nc.declare_dram_parameter("w1", [HT, 128, M], bf16, isOutput=False)
    # w2[ht, p, n] = fc2_w[e][ht*128+p, n]
    w2 = nc.declare_dram_parameter("w2", [HT, 128, M], bf16, isOutput=False)
    # b1[p, ht] = fc1_b[e][ht*128+p]
    b1 = nc.declare_dram_parameter("b1", [128, HT], f32, isOutput=False)
    y = nc.declare_dram_parameter("y", [c_pad, M], f32, isOutput=True)

    with tile.TileContext(nc) as tc:
        with (
            tc.tile_pool(name="w2res", bufs=1) as w2pool,
            tc.tile_pool(name="consts", bufs=1) as cpool,
            tc.tile_pool(name="w1s", bufs=12) as w1pool,
            tc.tile_pool(name="dt", bufs=2) as dtpool,
            tc.tile_pool(name="ht", bufs=2) as htpool,
            tc.tile_pool(name="ystage", bufs=3) as ypool,
            tc.tile_pool(name="ph", bufs=2, space="PSUM") as phpool,
            tc.tile_pool(name="py", bufs=4, space="PSUM") as pypool,
        ):
            b1_sb = cpool.tile([128, HT], f32, tag="b1")
            nc.sync.dma_start(out=b1_sb[:], in_=b1[:])

            # HAM warmup: the PE clock-gate defaults to 1.2 GHz and needs
            # ~3.4us of sustained activity to unthrottle. These dummy matmuls
            # run during the initial input-DMA wait so the first real matmuls
            # start at 2.4 GHz.
            wu = cpool.tile([128, 256], bf16, tag="wu")
            nc.vector.memset(wu[:], 0.0)
            with tc.tile_pool(name="wups", bufs=1, space="PSUM") as wupool:
                wups = wupool.tile([128, 256], f32, tag="wups")
                for _ in range(20):
                    nc.tensor.matmul(wups[:], lhsT=wu[:, :128],
                                     rhs=wu[:, :256], start=True, stop=True)

            # resident fc2 weights: 32 tiles of [128, 1024] bf16 (64KB/part).
            # Loads are emitted after chunk 0's GEMM1 (lower DMA priority than
            # the tiles the first matmuls need) — they fill DMA idle time
            # during chunk 0 compute and only gate chunk 0's GEMM2.
            w2_sb = [w2pool.tile([128, M], bf16, tag=f"w2_{ht}",
                                 name=f"w2sb_{ht}")
                     for ht in range(HT)]

            c0 = 0
            for cc, cch in enumerate(chunks):
                # this chunk's tokens, all 8 contraction tiles
                dt_sb = []
                for k in range(KT):
                    t = dtpool.tile([128, cch], bf16, tag=f"dt_{k}",
                                    name=f"dt_sb_{k}")
                    # chunk 0 goes on the scalar (ACT-HWDGE) queue so the
                    # first GEMM's inputs don't serialize behind w1 issue at
                    # startup; steady-state chunks stay on sync, which keeps
                    # the mid-run queues conflict-free
                    dma_eng = nc.scalar if cc == 0 else nc.sync
                    dma_eng.dma_start(
                        out=t[:], in_=dispT[k * 128:(k + 1) * 128, c0:c0 + cch])
                    dt_sb.append(t)

                # GEMM1: hT[ht] [128, cch] = relu(W1 @ dispT + b1)
                h_sb = []
                for ht in range(HT):
                    w1_sb = w1pool.tile([128, M], bf16, tag="w1")
                    nc.sync.dma_start(out=w1_sb[:], in_=w1[ht])
                    ph = phpool.tile([128, cch], f32, tag="ph")
                    for k in range(KT):
                        nc.tensor.matmul(
                            ph[:],
                            lhsT=w1_sb[:, k * 128:(k + 1) * 128],
                            rhs=dt_sb[k][:],
                            start=(k == 0),
                            stop=(k == KT - 1),
                        )
                    hh = htpool.tile([128, cch], bf16, tag=f"h_{ht}")
                    nc.scalar.activation(
                        out=hh[:], in_=ph[:],
                        func=mybir.ActivationFunctionType.Relu,
                        bias=b1_sb[:, ht:ht + 1], scale=1.0)
                    h_sb.append(hh)
                    if cc == 0 and ht >= 8:
                        # spread the resident-W2 loads across chunk 0's GEMM1
                        # so they fill DMA idle time without starving the w1
                        # stream; hold off until the w1 prefetch pipeline is
                        # deep enough
                        nc.sync.dma_start(out=w2_sb[ht - 8][:], in_=w2[ht - 8])
                        if ht >= 24:
                            nc.sync.dma_start(out=w2_sb[ht][:], in_=w2[ht])

                # GEMM2: y[c0+csub*128 : +128, nch*512 : +512]
                for nch in range(2):
                    for csub in range(cch // 128):
                        py = pypool.tile([128, 512], f32, tag="py")
                        for ht in range(HT):
                            nc.tensor.matmul(
                                py[:],
                                lhsT=h_sb[ht][:, csub * 128:(csub + 1) * 128],
                                rhs=w2_sb[ht][:, nch * 512:(nch + 1) * 512],
                                start=(ht == 0),
                                stop=(ht == HT - 1),
                            )
                        ys = ypool.tile([128, 512], f32, tag="ys")
                        nc.vector.tensor_copy(out=ys[:], in_=py[:])
                        nc.sync.dma_start(
                            out=y[c0 + csub * 128:c0 + (csub + 1) * 128,
                                  nch * 512:(nch + 1) * 512],
                            in_=ys[:])
                c0 += cch

    nc.compile()
    return nc


def _get_program(c_pad):
    if c_pad not in _PROGRAMS:
        _PROGRAMS[c_pad] = _build_program(c_pad)
    return _PROGRAMS[c_pad]


def _route(x, gate_w):
    """Exact GShard/Tutel k-major top-2 routing in numpy fp32."""
    logits = x @ gate_w  # [T, E]
    m = logits.max(axis=-1, keepdims=True)
    ex = np.exp(logits - m)
    gates = ex / ex.sum(axis=-1, keepdims=True)

    n = x.shape[0]
    ar = np.arange(n)
    e0 = np.argmax(gates, axis=-1)
    g0 = gates[ar, e0]
    gm = gates.copy()
    gm[ar, e0] = -np.inf
    e1 = np.argmax(gm, axis=-1)
    g1 = gates[ar, e1]
    s = g0 + g1
    g0, g1 = g0 / s, g1 / s

    e_flat = np.concatenate([e0, e1])  # k-major
    kt = e_flat.shape[0]
    sort_idx = np.argsort(e_flat, kind="stable")
    sorted_e = e_flat[sort_idx]
    first = np.r_[0, np.flatnonzero(np.diff(sorted_e)) + 1]
    counts = np.diff(np.r_[first, kt])
    grp_start = np.repeat(first, counts)
    pos = np.empty(kt, np.int64)
    pos[sort_idx] = np.arange(kt) - grp_start
    valid = pos < CAP
    slot = np.where(valid, e_flat * CAP + pos, 0)
    return e_flat, valid, slot, np.stack([g0, g1]), np.stack([e0, e1])


def kernel(x, gate_w, fc1_w, fc1_b, fc2_w, fc2_b):
    global LAST_RESULT
    from concourse.bass_utils import run_bass_kernel_spmd

    x = np.asarray(x, np.float32)
    gate_w = np.asarray(gate_w, np.float32)
    fc1_w = np.asarray(fc1_w, np.float32)
    fc1_b = np.asarray(fc1_b, np.float32)
    fc2_w = np.asarray(fc2_w, np.float32)
    fc2_b = np.asarray(fc2_b, np.float32)

    e_flat, valid, slot, g, top_e = _route(x, gate_w)

    # only rows [0, load_e) of each expert's capacity block are ever written
    # or gathered; size the compiled capacity to the max occupied slot
    # (rounded to the 256-token chunk granularity)
    pos = slot - e_flat * CAP  # position within expert (valid entries)
    max_pos = int(pos[valid].max()) if valid.any() else 0
    c_pad = min(CAP, ((max_pos + 1 + 255) // 256) * 256)

    # dispatch: pack selected token rows into [E*c_pad, M]
    disp = np.zeros((E, c_pad, M), np.float32)
    tok = np.tile(np.arange(T), K)
    ef_v, pos_v = e_flat[valid], pos[valid]
    disp[ef_v, pos_v] = x[tok[valid]]

    bf = ml_dtypes.bfloat16
    in_maps = []
    for e in range(E):
        dispT_e = np.ascontiguousarray(disp[e].T).astype(bf)
        w1_e = np.ascontiguousarray(
            fc1_w[e].reshape(HT, 128, KT, 128).transpose(0, 3, 2, 1)
        ).reshape(HT, 128, M).astype(bf)
        w2_e = fc2_w[e].reshape(HT, 128, M).astype(bf)
        b1_e = np.ascontiguousarray(fc1_b[e].reshape(HT, 128).T)
        in_maps.append({"dispT": dispT_e, "w1": w1_e, "w2": w2_e, "b1": b1_e})

    nc = _get_program(c_pad)
    res = run_bass_kernel_spmd(nc, in_maps, core_ids=list(range(N_CORES)),
                               trace=PROFILE)
    LAST_RESULT = res

    y_flat = np.concatenate([res.results[e]["y"] for e in range(E)], axis=0)

    # combine: weighted gather + fc2 bias contribution
    validK = valid.reshape(K, T)
    eK = e_flat.reshape(K, T)
    posK = np.where(valid, pos, 0).reshape(K, T)
    gv = (g * validK).astype(np.float32)
    out = np.zeros((T, M), np.float32)
    for k in range(K):
        idx = eK[k] * c_pad + posK[k]
        contrib = y_flat[idx] * gv[k][:, None]
        out += np.where(validK[k][:, None], contrib, 0.0)
        out += gv[k][:, None] * fc2_b[top_e[k]]
    return out

